# revision 11
# baseline (speedup 1.0000x reference)
"""AttentionBlock (GroupNorm -> 1x1 qkv -> MHA -> 1x1 proj -> residual)
for Trainium2, SPMD over 8 NeuronCores.

Problem shapes (hardcoded): x [4, 256, 64, 64] fp32, NUM_HEADS=4 (hd=64),
NUM_GROUPS=8, qkv_w [768, 256], proj_w [256, 256], eps=1e-5.

Sharding: core c handles (batch b = c//2, query-pixel half = c%2).
Each core group-norms its full batch slab [256, 4096], computes K/V over
all 4096 pixels, Q/attention/proj over its 2048 query pixels. The host
rotates each core's x slab so its own query pixels are columns 0..2048
(attention is invariant to kv-pixel permutation), so one SPMD program
serves all cores; outputs are disjoint column halves, concatenated on
host. No collectives.

Numerics: all matmuls run as float32r (TF32-like, 1 cycle/row at N>=512
vs 4 for fp32); softmax skips max-subtraction (|S*scale| <~ 8 so exp is
safe) which lets ACT do exp straight out of PSUM; softmax denominators
come free from an appended ones-column in the PV stationary operand.
Attention is computed in the S^T orientation (kv index on partitions) so
no P-matrix transposes are needed anywhere.
"""
import os
import sys

for _p in ("/root/.axon_site/_ro/trn_rl_repo", "/opt/trn_rl_repo"):
    if os.path.isdir(_p) and _p not in sys.path:
        sys.path.append(_p)

import numpy as np
import concourse.bass as bass
import concourse.tile as tile
import concourse.mybir as mybir
from concourse.bass_utils import run_bass_kernel_spmd
from concourse.masks import make_identity

F32 = mybir.dt.float32
F32R = mybir.dt.float32r
BF16 = mybir.dt.bfloat16
AF = mybir.ActivationFunctionType
OP = mybir.AluOpType

C = 256            # channels
N = 4096           # pixels (64*64)
B = 4              # batch
NQ = 2048          # query pixels per core
NH = 4             # heads
HD = 64            # head dim
G = 8              # groups
GS = C // G        # 32 channels per group
EPS = 1e-5
SCALE = HD ** -0.5
NCORES = 8

_CACHE = {}


def _build(dbg=False):
    nc = bass.Bass("TRN2", target_bir_lowering=False, debug=False,
                   num_devices=NCORES)
    x_in = nc.dram_tensor("x", [C, N], F32, kind="ExternalInput").ap()
    qw_in = nc.dram_tensor("qkv_w", [3 * C, C], F32, kind="ExternalInput").ap()
    qb_in = nc.dram_tensor("qkv_b", [3 * C, 1], F32, kind="ExternalInput").ap()
    pw_in = nc.dram_tensor("proj_w", [C, C], F32, kind="ExternalInput").ap()
    pb_in = nc.dram_tensor("proj_b", [C, 1], F32, kind="ExternalInput").ap()
    nw_in = nc.dram_tensor("norm_w", [C, 1], F32, kind="ExternalInput").ap()
    nb_in = nc.dram_tensor("norm_b", [C, 1], F32, kind="ExternalInput").ap()
    gi_in = nc.dram_tensor("gind", [2, 8, 128], F32, kind="ExternalInput").ap()
    git_in = nc.dram_tensor("gind_t", [2, 128, 8], F32, kind="ExternalInput").ap()
    out_d = nc.dram_tensor("out", [C, NQ], F32, kind="ExternalOutput").ap()
    if dbg:
        dbg_d = {
            "d_gstat": nc.dram_tensor("d_gstat", [8, 2], F32, kind="ExternalOutput").ap(),
            "d_grstd": nc.dram_tensor("d_grstd", [8, 1], F32, kind="ExternalOutput").ap(),
            "d_h0": nc.dram_tensor("d_h0", [128, N], F32, kind="ExternalOutput").ap(),
            "d_Q0": nc.dram_tensor("d_Q0", [128, NQ], F32, kind="ExternalOutput").ap(),
            "d_K0": nc.dram_tensor("d_K0", [128, N], F32, kind="ExternalOutput").ap(),
            "d_vta0": nc.dram_tensor("d_vta0", [128, 32 * 130], F32, kind="ExternalOutput").ap(),
            "d_attn0": nc.dram_tensor("d_attn0", [128, NQ], F32, kind="ExternalOutput").ap(),
        }

    with (
        nc.allow_low_precision(reason="fp32r matmul pipeline"),
        tile.TileContext(nc) as tc,
        tc.tile_pool(name="per", bufs=1) as per,
        tc.tile_pool(name="work3", bufs=3) as work3,
        tc.tile_pool(name="work2", bufs=2) as work2,
        tc.tile_pool(name="psA", bufs=2, space="PSUM") as psA,
        tc.tile_pool(name="psB", bufs=2, space="PSUM") as psB,
    ):
        # ---------- constants / small loads ----------
        with nc.allow_non_contiguous_dma(reason="tiny bias loads"):
            qkvb = per.tile([128, 6], F32, tag="qkvb")
            nc.sync.dma_start(out=qkvb[:], in_=qb_in.rearrange("(t p) o -> p (t o)", p=128))
            projb = per.tile([128, 2], F32, tag="projb")
            nc.sync.dma_start(out=projb[:], in_=pb_in.rearrange("(t p) o -> p (t o)", p=128))
            normw = per.tile([128, 2], F32, tag="normw")
            nc.sync.dma_start(out=normw[:], in_=nw_in.rearrange("(t p) o -> p (t o)", p=128))
            normb = per.tile([128, 2], F32, tag="normb")
            nc.sync.dma_start(out=normb[:], in_=nb_in.rearrange("(t p) o -> p (t o)", p=128))
            vbias_f = per.tile([1, C], F32, tag="vbias_f")
            nc.sync.dma_start(out=vbias_f[:], in_=qb_in[2 * C:3 * C, :].rearrange("o u -> u o"))

        ident = per.tile([128, 128], F32, tag="ident")
        make_identity(nc, ident)
        ones_col = per.tile([128, 1], F32, tag="ones_col")
        nc.vector.memset(ones_col[:], 1.0)
        ones_f = per.tile([1, 128], F32, tag="ones_f")
        nc.vector.memset(ones_f[:], 1.0)
        ones1r = per.tile([1, 128], F32R, tag="ones1r")
        nc.vector.tensor_copy(out=ones1r[:], in_=ones_f[:])
        vbias_r = per.tile([1, C], F32R, tag="vbias_r")
        nc.vector.tensor_copy(out=vbias_r[:], in_=vbias_f[:])
        ones1b = per.tile([1, 128], BF16, tag="ones1b")
        nc.vector.tensor_copy(out=ones1b[:], in_=ones_f[:])
        vbias_b = per.tile([1, C], BF16, tag="vbias_b")
        nc.vector.tensor_copy(out=vbias_b[:], in_=vbias_f[:])
        eps8 = per.tile([8, 1], F32, tag="eps8")
        nc.vector.memset(eps8[:], EPS)
        # group indicators (host constants): g0t[blk] [8, 128] and its
        # transpose gsb[blk] [128, 8]
        g0t = [per.tile([8, 128], F32, tag=f"g0t{b_}", name=f"g0t{b_}")
               for b_ in range(2)]
        gsb = [per.tile([128, 8], F32, tag=f"gsb{b_}", name=f"gsb{b_}")
               for b_ in range(2)]
        for blk in range(2):
            nc.sync.dma_start(out=g0t[blk][:], in_=gi_in[blk])
            nc.sync.dma_start(out=gsb[blk][:], in_=git_in[blk])

        # persistent big tiles
        xq = [per.tile([128, NQ], F32, tag=f"xq{b_}", name=f"xq{b_}")
              for b_ in range(2)]
        Q = [per.tile([128, NQ], BF16, tag=f"Q{hp}", name=f"Q{hp}") for hp in range(2)]
        K = [per.tile([128, N], BF16, tag=f"K{hp}", name=f"K{hp}") for hp in range(2)]
        vta = [per.tile([128, 32, 130], BF16, tag=f"vta{hp}", name=f"vta{hp}")
               for hp in range(2)]
        wtq = [per.tile([128, 6 * 128], BF16, tag=f"wtq{b_}", name=f"wtq{b_}")
               for b_ in range(2)]
        wtp = [per.tile([128, 2 * 128], F32R, tag=f"wtp{b_}", name=f"wtp{b_}")
               for b_ in range(2)]

        with tc.tile_pool(name="tmp", bufs=1) as tmp:
            # h blocks + weight staging live only through qkv/V^T
            wstage = tmp.tile([128, 6, C], F32, tag="wstage")
            nc.sync.dma_start(out=wstage[:], in_=qw_in.rearrange("(t p) c -> p t c", p=128))
            pstage = tmp.tile([128, 2, C], F32, tag="pstage")
            nc.sync.dma_start(out=pstage[:], in_=pw_in.rearrange("(t p) c -> p t c", p=128))
            h = [tmp.tile([128, N], BF16, tag=f"h{b_}", name=f"h{b_}")
                 for b_ in range(2)]

            with tc.tile_pool(name="xkp", bufs=1) as xkp:
                xk = [xkp.tile([128, NQ], F32, tag=f"xk{b_}", name=f"xk{b_}")
                      for b_ in range(2)]
                for blk in range(2):
                    rsl = slice(blk * 128, (blk + 1) * 128)
                    nc.sync.dma_start(out=xq[blk][:], in_=x_in[rsl, 0:NQ])
                    nc.sync.dma_start(out=xk[blk][:], in_=x_in[rsl, NQ:N])

                # ---------- group norm ----------
                stats2 = []
                for blk in range(2):
                    st = work2.tile([128, 8, 6], F32, tag="bnst", name=f"bnst{blk}")
                    for sg in range(4):
                        nc.vector.bn_stats(out=st[:, sg, :],
                                           in_=xq[blk][:, sg * 512:(sg + 1) * 512])
                        nc.vector.bn_stats(out=st[:, 4 + sg, :],
                                           in_=xk[blk][:, sg * 512:(sg + 1) * 512])
                    mv = work2.tile([128, 2], F32, tag="bnmv", name=f"bnmv{blk}")
                    nc.vector.bn_aggr(out=mv[:], in_=st[:])
                    s2 = per.tile([128, 2], F32, tag=f"s2_{blk}", name=f"s2_{blk}")
                    nc.vector.tensor_copy(out=s2[:, 0:1], in_=mv[:, 0:1])
                    msq = work2.tile([128, 1], F32, tag="msq", name=f"msq{blk}")
                    nc.vector.tensor_mul(out=msq[:], in0=mv[:, 0:1], in1=mv[:, 0:1])
                    nc.vector.tensor_add(out=s2[:, 1:2], in0=mv[:, 1:2], in1=msq[:])
                    stats2.append(s2)

                gstat_ps = psA.tile([8, 2, 512], F32, tag="s", name="gstat_ps")
                for blk in range(2):
                    nc.tensor.matmul(gstat_ps[:, 0, 0:2], gsb[blk][:], stats2[blk][:],
                                     start=(blk == 0), stop=(blk == 1))
                gstat = per.tile([8, 2], F32, tag="gstat")
                nc.vector.tensor_scalar_mul(out=gstat[:], in0=gstat_ps[:, 0, 0:2],
                                            scalar1=1.0 / GS)
                gmsq = per.tile([8, 1], F32, tag="gmsq")
                nc.vector.tensor_mul(out=gmsq[:], in0=gstat[:, 0:1], in1=gstat[:, 0:1])
                gvar = per.tile([8, 1], F32, tag="gvar")
                nc.vector.tensor_tensor(out=gvar[:], in0=gstat[:, 1:2], in1=gmsq[:],
                                        op=OP.subtract)
                # rstd = exp(-0.5 * ln(var + eps)); Ln+Exp share one ACT table set
                glog = per.tile([8, 1], F32, tag="glog")
                nc.scalar.activation(out=glog[:], in_=gvar[:], func=AF.Ln,
                                     bias=eps8[:], scale=1.0)
                grstd = per.tile([8, 1], F32, tag="grstd")
                nc.scalar.activation(out=grstd[:], in_=glog[:], func=AF.Exp, scale=-0.5)
                ms8 = per.tile([8, 2], F32, tag="ms8")
                nc.vector.tensor_copy(out=ms8[:, 0:1], in_=gstat[:, 0:1])
                nc.vector.tensor_copy(out=ms8[:, 1:2], in_=grstd[:])

                for blk in range(2):
                    bc_ps = psA.tile([128, 2, 512], F32, tag="s", name=f"gnbc{blk}")
                    nc.tensor.matmul(bc_ps[:, 0, 0:2], g0t[blk][:], ms8[:],
                                     start=True, stop=True)
                    A = work2.tile([128, 1], F32, tag="gnA", name=f"gnA{blk}")
                    nc.vector.tensor_mul(out=A[:], in0=bc_ps[:, 0, 1:2],
                                         in1=normw[:, blk:blk + 1])
                    t0 = work2.tile([128, 1], F32, tag="gnT", name=f"gnT{blk}")
                    nc.vector.tensor_mul(out=t0[:], in0=bc_ps[:, 0, 0:1], in1=A[:])
                    Bc = work2.tile([128, 1], F32, tag="gnB", name=f"gnB{blk}")
                    nc.vector.tensor_tensor(out=Bc[:], in0=normb[:, blk:blk + 1],
                                            in1=t0[:], op=OP.subtract)
                    nc.vector.tensor_scalar(out=h[blk][:, 0:NQ], in0=xq[blk][:],
                                            scalar1=A[:], scalar2=Bc[:],
                                            op0=OP.mult, op1=OP.add)
                    nc.vector.tensor_scalar(out=h[blk][:, NQ:N], in0=xk[blk][:],
                                            scalar1=A[:], scalar2=Bc[:],
                                            op0=OP.mult, op1=OP.add)

            # ---------- weight transposes (no x dependency; PE warms early) ----------
            for cb in range(2):
                for t in range(6):
                    tp = psA.tile([128, 2, 512], F32, tag="s", name=f"wq{cb}_{t}")
                    nc.tensor.transpose(tp[:, 0, 0:128],
                                        wstage[:, t, cb * 128:(cb + 1) * 128], ident[:])
                    nc.vector.tensor_copy(out=wtq[cb][:, t * 128:(t + 1) * 128],
                                          in_=tp[:, 0, 0:128])
                for t in range(2):
                    tp = psA.tile([128, 2, 512], F32, tag="s", name=f"wp{cb}_{t}")
                    nc.tensor.transpose(tp[:, 0, 0:128],
                                        pstage[:, t, cb * 128:(cb + 1) * 128], ident[:])
                    nc.vector.tensor_copy(out=wtp[cb][:, t * 128:(t + 1) * 128],
                                          in_=tp[:, 0, 0:128])

            # ---------- qkv / V^T emission helpers ----------
            def q_chunk(hp, ch):
                qp = psA.tile([128, 2, 512], F32, tag="s", name=f"qp{hp}_{ch}")
                for cb in range(2):
                    nc.tensor.matmul(qp[:, 0], wtq[cb][:, hp * 128:(hp + 1) * 128],
                                     h[cb][:, ch * 512:(ch + 1) * 512],
                                     start=(cb == 0), stop=(cb == 1))
                nc.vector.tensor_scalar(out=Q[hp][:, ch * 512:(ch + 1) * 512],
                                        in0=qp[:, 0], scalar1=qkvb[:, hp:hp + 1],
                                        scalar2=None, op0=OP.add)

            def k_chunk(hp, ch):
                kp = psA.tile([128, 2, 512], F32, tag="s", name=f"kp{hp}_{ch}")
                for cb in range(2):
                    nc.tensor.matmul(kp[:, 0],
                                     wtq[cb][:, 256 + hp * 128:256 + (hp + 1) * 128],
                                     h[cb][:, ch * 512:(ch + 1) * 512],
                                     start=(cb == 0), stop=(cb == 1))
                nc.vector.tensor_scalar(out=K[hp][:, ch * 512:(ch + 1) * 512],
                                        in0=kp[:, 0], scalar1=qkvb[:, 2 + hp:3 + hp],
                                        scalar2=None, op0=OP.add)

            def vta_tile(hp, j):
                vp = psA.tile([128, 2, 512], F32, tag="s", name=f"vp{hp}_{j}")
                nc.tensor.matmul(vp[:, 0, 0:128], ones1b[:],
                                 vbias_b[:, hp * 128:(hp + 1) * 128],
                                 start=True, stop=False)
                for cb in range(2):
                    nc.tensor.matmul(vp[:, 0, 0:128], h[cb][:, j * 128:(j + 1) * 128],
                                     wtq[cb][:, 512 + hp * 128:512 + (hp + 1) * 128],
                                     start=False, stop=(cb == 1))
                nc.vector.tensor_copy(out=vta[hp][:, j, 0:64], in_=vp[:, 0, 0:64])
                nc.vector.tensor_copy(out=vta[hp][:, j, 65:129], in_=vp[:, 0, 64:128])

            # ones columns for the softmax-denominator trick (no data deps)
            for hp in range(2):
                nc.vector.tensor_copy(out=vta[hp][:, :, 64:65],
                                      in_=ones_col[:, 0:1, None].to_broadcast((128, 32, 1)))
                nc.vector.tensor_copy(out=vta[hp][:, :, 129:130],
                                      in_=ones_col[:, 0:1, None].to_broadcast((128, 32, 1)))

            # hp=0 prep upfront; hp=1 prep + vta tiles stream in as
            # background work inside the attention loops
            for ch in range(NQ // 512):
                q_chunk(0, ch)
            for ch in range(N // 512):
                k_chunk(0, ch)

            # ---------- attention ----------
            attn = [per.tile([128, NQ], F32R, tag=f"attn{hp}", name=f"attn{hp}")
                    for hp in range(2)]

            def attn_recips(hp, qc, outp):
                rcs = []
                for h_ in range(2):
                    rc = work2.tile([1, 512], F32R, tag="rc", name=f"rc{hp}{qc}{h_}")
                    nc.vector.reciprocal(out=rc[:], in_=outp[h_][64:65, :])
                    rcs.append(rc)
                return rcs

            def attn_finish(hp, qc, outp, rcs):
                qsl = slice(qc * 512, (qc + 1) * 512)
                for h_ in range(2):
                    bcp = psA.tile([128, 2, 512], F32, tag="s", name=f"bcp{hp}{qc}{h_}")
                    nc.tensor.matmul(bcp[0:64, 0], ones1r[:, 0:64], rcs[h_][:],
                                     start=True, stop=True)
                    bcs = work2.tile([64, 512], F32, tag="bcs", name=f"bcs{hp}{qc}{h_}")
                    nc.vector.tensor_copy(out=bcs[:], in_=bcp[0:64, 0])
                    nc.vector.tensor_tensor(out=attn[hp][h_ * 64:(h_ + 1) * 64, qsl],
                                            in0=outp[h_][0:64, :], in1=bcs[:],
                                            op=OP.mult)

            def proj_chunk(ch):
                for ob in range(2):
                    pj = psB.tile([128, 512], F32, tag=f"out{ob}", name=f"pj{ob}_{ch}")
                    for cb in range(2):
                        nc.tensor.matmul(pj[:], wtp[cb][:, ob * 128:(ob + 1) * 128],
                                         attn[cb][:, ch * 512:(ch + 1) * 512],
                                         start=(cb == 0), stop=(cb == 1))
                    osb = work2.tile([128, 512], F32, tag="osb", name=f"osb{ob}_{ch}")
                    nc.vector.tensor_scalar(out=osb[:], in0=pj[:],
                                            scalar1=projb[:, ob:ob + 1],
                                            scalar2=None, op0=OP.add)
                    nc.vector.tensor_add(out=osb[:], in0=osb[:],
                                         in1=xq[ob][:, ch * 512:(ch + 1) * 512])
                    nc.sync.dma_start(out=out_d[ob * 128:(ob + 1) * 128,
                                                ch * 512:(ch + 1) * 512],
                                      in_=osb[:])

            bg = [lambda j=j: vta_tile(0, j) for j in range(32)]
            bg2 = ([lambda c=c: q_chunk(1, c) for c in range(NQ // 512)]
                   + [lambda c=c: k_chunk(1, c) for c in range(N // 512)]
                   + [lambda j=j: vta_tile(1, j) for j in range(32)])
            pending = None
            for hp in range(2):
                for qc in range(NQ // 512):
                    qsl = slice(qc * 512, (qc + 1) * 512)
                    outp = [psB.tile([128, 512], F32, tag=f"out{h_}",
                                     name=f"o{hp}{qc}{h_}")
                            for h_ in range(2)]
                    pbs = [None] * 32
                    for j in range(33):
                        if bg:
                            bg.pop(0)()
                        if j == 8 and pending is not None:
                            attn_finish(*pending)
                            pending = None
                        if j == 16 and hp == 1 and qc >= 1:
                            proj_chunk(qc - 1)
                        if j < 32:
                            sp = psA.tile([128, 2, 512], F32, tag="s",
                                          name=f"sp{hp}_{qc}_{j}")
                            nc.tensor.matmul(sp[:, 0], K[hp][0:64, j * 128:(j + 1) * 128],
                                             Q[hp][0:64, qsl], start=True, stop=True,
                                             tile_position=(0, 0))
                            nc.tensor.matmul(sp[:, 1], K[hp][64:128, j * 128:(j + 1) * 128],
                                             Q[hp][64:128, qsl], start=True, stop=True,
                                             tile_position=(64, 0))
                            pb = work3.tile([128, 2, 512], BF16, tag="p",
                                            name=f"pb{hp}_{qc}_{j}")
                            nc.scalar.activation(out=pb[:], in_=sp[:], func=AF.Exp,
                                                 scale=SCALE)
                            pbs[j] = pb
                        if j > 0:
                            nc.tensor.matmul(outp[0][0:65], vta[hp][:, j - 1, 0:65],
                                             pbs[j - 1][:, 0],
                                             start=(j == 1), stop=(j == 32))
                            nc.tensor.matmul(outp[1][0:65], vta[hp][:, j - 1, 65:130],
                                             pbs[j - 1][:, 1],
                                             start=(j == 1), stop=(j == 32))
                    pending = (hp, qc, outp, attn_recips(hp, qc, outp))
                    if hp == 0 and qc == 0:
                        bg = bg2
            attn_finish(*pending)
            proj_chunk(NQ // 512 - 1)

            if dbg:
                nc.sync.dma_start(out=dbg_d["d_h0"], in_=h[0][:].bitcast(mybir.dt.uint16))
                nc.sync.dma_start(out=dbg_d["d_Q0"], in_=Q[0][:].bitcast(mybir.dt.uint16))
                nc.sync.dma_start(out=dbg_d["d_K0"], in_=K[0][:].bitcast(mybir.dt.uint16))
                nc.sync.dma_start(out=dbg_d["d_attn0"], in_=attn[0][:].bitcast(F32))

    _split_waits(nc)
    return nc


def _split_waits(nc):
    """walrus in this container caps sync waits at 1/instruction; hoist
    extras onto preceding single-wait NOPs on the same engine."""
    n_new = 0
    for f in nc.m.functions:
        for bb in f.blocks:
            out = []
            changed = False
            for inst in bb.instructions:
                si = inst.sync_info
                if si is not None and si.on_wait and len(si.on_wait) > 1:
                    waits = list(si.on_wait)
                    for w in waits[:-1]:
                        nop = mybir.InstNoOp(
                            name=f"{inst.name}-ws{n_new}", engine=inst.engine,
                            ins=[], outs=[],
                            sync_info=mybir.SyncInfo(on_wait=[w], on_update=[]),
                        )
                        n_new += 1
                        out.append(nop)
                    si.on_wait = [waits[-1]]
                    changed = True
                out.append(inst)
            if changed:
                bb.instructions = out
    return n_new


def _get_nc(dbg=False):
    key = ("nc", dbg)
    if key not in _CACHE:
        _CACHE[key] = _build(dbg)
    return _CACHE[key]


def _gind():
    g = np.zeros((2, 8, 128), dtype=np.float32)
    for blk in range(2):
        for gr in range(4):
            g[blk, blk * 4 + gr, gr * GS:(gr + 1) * GS] = 1.0
    return g


def _core_maps(x, norm_w, norm_b, qkv_w, qkv_b, proj_w, proj_b):
    x = np.ascontiguousarray(np.asarray(x, dtype=np.float32)).reshape(B, C, N)
    maps = []
    shared = {
        "qkv_w": np.ascontiguousarray(np.asarray(qkv_w, dtype=np.float32)),
        "qkv_b": np.asarray(qkv_b, dtype=np.float32).reshape(3 * C, 1),
        "proj_w": np.ascontiguousarray(np.asarray(proj_w, dtype=np.float32)),
        "proj_b": np.asarray(proj_b, dtype=np.float32).reshape(C, 1),
        "norm_w": np.asarray(norm_w, dtype=np.float32).reshape(C, 1),
        "norm_b": np.asarray(norm_b, dtype=np.float32).reshape(C, 1),
        "gind": _gind(),
        "gind_t": np.ascontiguousarray(_gind().transpose(0, 2, 1)),
    }
    for core in range(NCORES):
        b, half = core // 2, core % 2
        xb = x[b]
        if half == 0:
            xc = xb
        else:
            xc = np.concatenate([xb[:, NQ:], xb[:, :NQ]], axis=1)
        m = dict(shared)
        m["x"] = np.ascontiguousarray(xc)
        maps.append(m)
    return maps


def _run(inputs, trace=False, dbg=False):
    nc = _get_nc(dbg)
    maps = _core_maps(**inputs)
    res = run_bass_kernel_spmd(nc, maps, list(range(NCORES)), trace=trace)
    full = np.empty((B, C, N), dtype=np.float32)
    for core in range(NCORES):
        b, half = core // 2, core % 2
        full[b][:, half * NQ:(half + 1) * NQ] = res.results[core]["out"]
    return full.reshape(B, C, 64, 64), res


def kernel(**inputs):
    out, _ = _run(inputs, trace=False)
    return out


# revision 16
# speedup vs baseline: 1.3215x; 1.3215x over previous
"""AttentionBlock (GroupNorm -> 1x1 qkv -> MHA -> 1x1 proj -> residual)
for Trainium2, SPMD over 8 NeuronCores.

Problem shapes (hardcoded): x [4, 256, 64, 64] fp32, NUM_HEADS=4 (hd=64),
NUM_GROUPS=8, qkv_w [768, 256], proj_w [256, 256], eps=1e-5.

Sharding: core c handles (batch b = c//2, query-pixel half = c%2).
Each core group-norms its full batch slab [256, 4096], computes K/V over
all 4096 pixels, Q/attention/proj over its 2048 query pixels. The host
rotates each core's x slab so its own query pixels are columns 0..2048
(attention is invariant to kv-pixel permutation), so one SPMD program
serves all cores; outputs are disjoint column halves, concatenated on
host. No collectives.

Numerics: all matmuls run as float32r (TF32-like, 1 cycle/row at N>=512
vs 4 for fp32); softmax skips max-subtraction (|S*scale| <~ 8 so exp is
safe) which lets ACT do exp straight out of PSUM; softmax denominators
come free from an appended ones-column in the PV stationary operand.
Attention is computed in the S^T orientation (kv index on partitions) so
no P-matrix transposes are needed anywhere.
"""
import os
import sys

for _p in ("/root/.axon_site/_ro/trn_rl_repo", "/opt/trn_rl_repo"):
    if os.path.isdir(_p) and _p not in sys.path:
        sys.path.append(_p)

import numpy as np
import concourse.bass as bass
import concourse.tile as tile
import concourse.mybir as mybir
from concourse.bass_utils import run_bass_kernel_spmd
from concourse.masks import make_identity

F32 = mybir.dt.float32
F32R = mybir.dt.float32r
BF16 = mybir.dt.bfloat16
AF = mybir.ActivationFunctionType
OP = mybir.AluOpType

C = 256            # channels
N = 4096           # pixels (64*64)
B = 4              # batch
NQ = 2048          # query pixels per core
NH = 4             # heads
HD = 64            # head dim
G = 8              # groups
GS = C // G        # 32 channels per group
EPS = 1e-5
SCALE = HD ** -0.5
NCORES = 8

_CACHE = {}


def _build(dbg=False):
    nc = bass.Bass("TRN2", target_bir_lowering=False, debug=False,
                   num_devices=NCORES)
    x_in = nc.dram_tensor("x", [C, N], F32, kind="ExternalInput").ap()
    qw_in = nc.dram_tensor("qkv_w", [3 * C, C], F32, kind="ExternalInput").ap()
    qb_in = nc.dram_tensor("qkv_b", [3 * C, 1], F32, kind="ExternalInput").ap()
    pw_in = nc.dram_tensor("proj_w", [C, C], F32, kind="ExternalInput").ap()
    pb_in = nc.dram_tensor("proj_b", [C, 1], F32, kind="ExternalInput").ap()
    nw_in = nc.dram_tensor("norm_w", [C, 1], F32, kind="ExternalInput").ap()
    nb_in = nc.dram_tensor("norm_b", [C, 1], F32, kind="ExternalInput").ap()
    gi_in = nc.dram_tensor("gind", [2, 8, 128], F32, kind="ExternalInput").ap()
    git_in = nc.dram_tensor("gind_t", [2, 128, 8], F32, kind="ExternalInput").ap()
    ei_in = nc.dram_tensor("eind", [8, 8 * 128], F32, kind="ExternalInput").ap()
    out_d = nc.dram_tensor("out", [C, NQ], F32, kind="ExternalOutput").ap()
    if dbg:
        dbg_d = {
            "d_gstat": nc.dram_tensor("d_gstat", [8, 2], F32, kind="ExternalOutput").ap(),
            "d_grstd": nc.dram_tensor("d_grstd", [8, 1], F32, kind="ExternalOutput").ap(),
            "d_h0": nc.dram_tensor("d_h0", [128, N], F32, kind="ExternalOutput").ap(),
            "d_Q0": nc.dram_tensor("d_Q0", [128, NQ], F32, kind="ExternalOutput").ap(),
            "d_K0": nc.dram_tensor("d_K0", [128, N], F32, kind="ExternalOutput").ap(),
            "d_vta0": nc.dram_tensor("d_vta0", [128, 32 * 130], F32, kind="ExternalOutput").ap(),
            "d_attn0": nc.dram_tensor("d_attn0", [128, NQ], F32, kind="ExternalOutput").ap(),
        }

    with (
        nc.allow_low_precision(reason="fp32r matmul pipeline"),
        tile.TileContext(nc) as tc,
        tc.tile_pool(name="per", bufs=1) as per,
        tc.tile_pool(name="work3", bufs=3) as work3,
        tc.tile_pool(name="work2", bufs=2) as work2,
        tc.tile_pool(name="psA", bufs=2, space="PSUM") as psA,
        tc.tile_pool(name="psB", bufs=1, space="PSUM") as psB,
        tc.tile_pool(name="psC", bufs=2, space="PSUM") as psC,
    ):
        # ---------- constants / small loads ----------
        with nc.allow_non_contiguous_dma(reason="tiny bias loads"):
            qkvb = per.tile([128, 6], F32, tag="qkvb")
            nc.sync.dma_start(out=qkvb[:], in_=qb_in.rearrange("(t p) o -> p (t o)", p=128))
            projb = per.tile([128, 2], F32, tag="projb")
            nc.sync.dma_start(out=projb[:], in_=pb_in.rearrange("(t p) o -> p (t o)", p=128))
            normw = per.tile([128, 2], F32, tag="normw")
            nc.sync.dma_start(out=normw[:], in_=nw_in.rearrange("(t p) o -> p (t o)", p=128))
            normb = per.tile([128, 2], F32, tag="normb")
            nc.sync.dma_start(out=normb[:], in_=nb_in.rearrange("(t p) o -> p (t o)", p=128))
            vbias_f = per.tile([1, C], F32, tag="vbias_f")
            nc.sync.dma_start(out=vbias_f[:], in_=qb_in[2 * C:3 * C, :].rearrange("o u -> u o"))

        ident = per.tile([128, 128], F32, tag="ident")
        make_identity(nc, ident)
        ones_col = per.tile([128, 1], F32, tag="ones_col")
        nc.vector.memset(ones_col[:], 1.0)
        ones_f = per.tile([1, 128], F32, tag="ones_f")
        nc.vector.memset(ones_f[:], 1.0)
        ones1r = per.tile([1, 128], F32R, tag="ones1r")
        nc.vector.tensor_copy(out=ones1r[:], in_=ones_f[:])
        vbias_r = per.tile([1, C], F32R, tag="vbias_r")
        nc.vector.tensor_copy(out=vbias_r[:], in_=vbias_f[:])
        ones1b = per.tile([1, 128], BF16, tag="ones1b")
        nc.vector.tensor_copy(out=ones1b[:], in_=ones_f[:])
        vbias_b = per.tile([1, C], BF16, tag="vbias_b")
        nc.vector.tensor_copy(out=vbias_b[:], in_=vbias_f[:])
        eind_f = per.tile([8, 8 * 128], F32, tag="eind_f")
        nc.sync.dma_start(out=eind_f[:], in_=ei_in[:])
        eind_r = per.tile([8, 8 * 128], F32R, tag="eind_r")
        nc.vector.tensor_copy(out=eind_r[:], in_=eind_f[:])
        eps8 = per.tile([8, 1], F32, tag="eps8")
        nc.vector.memset(eps8[:], EPS)
        # group indicators (host constants): g0t[blk] [8, 128] and its
        # transpose gsb[blk] [128, 8]
        g0t = [per.tile([8, 128], F32, tag=f"g0t{b_}", name=f"g0t{b_}")
               for b_ in range(2)]
        gsb = [per.tile([128, 8], F32, tag=f"gsb{b_}", name=f"gsb{b_}")
               for b_ in range(2)]
        for blk in range(2):
            nc.sync.dma_start(out=g0t[blk][:], in_=gi_in[blk])
            nc.sync.dma_start(out=gsb[blk][:], in_=git_in[blk])

        # persistent big tiles
        xq = [per.tile([128, NQ], F32, tag=f"xq{b_}", name=f"xq{b_}")
              for b_ in range(2)]
        Q = [per.tile([128, NQ], BF16, tag=f"Q{hp}", name=f"Q{hp}") for hp in range(2)]
        K = [per.tile([128, N], BF16, tag=f"K{hp}", name=f"K{hp}") for hp in range(2)]
        vta = [per.tile([128, 32, 130], BF16, tag=f"vta{hp}", name=f"vta{hp}")
               for hp in range(2)]
        wtq = [per.tile([128, 6 * 128], BF16, tag=f"wtq{b_}", name=f"wtq{b_}")
               for b_ in range(2)]
        wtp = [per.tile([128, 2 * 128], F32R, tag=f"wtp{b_}", name=f"wtp{b_}")
               for b_ in range(2)]

        with tc.tile_pool(name="tmp", bufs=1) as tmp:
            # h blocks + weight staging live only through qkv/V^T
            wstage = tmp.tile([128, 6, C], F32, tag="wstage")
            nc.sync.dma_start(out=wstage[:], in_=qw_in.rearrange("(t p) c -> p t c", p=128))
            pstage = tmp.tile([128, 2, C], F32, tag="pstage")
            nc.sync.dma_start(out=pstage[:], in_=pw_in.rearrange("(t p) c -> p t c", p=128))
            h = [tmp.tile([128, N], BF16, tag=f"h{b_}", name=f"h{b_}")
                 for b_ in range(2)]

            with tc.tile_pool(name="xkp", bufs=1) as xkp:
                xk = [xkp.tile([128, NQ], F32, tag=f"xk{b_}", name=f"xk{b_}")
                      for b_ in range(2)]
                for blk in range(2):
                    rsl = slice(blk * 128, (blk + 1) * 128)
                    nc.sync.dma_start(out=xq[blk][:], in_=x_in[rsl, 0:NQ])
                    nc.sync.dma_start(out=xk[blk][:], in_=x_in[rsl, NQ:N])

                # ---------- group norm ----------
                stats2 = []
                for blk in range(2):
                    st = work2.tile([128, 8, 6], F32, tag="bnst", name=f"bnst{blk}")
                    for sg in range(4):
                        nc.vector.bn_stats(out=st[:, sg, :],
                                           in_=xq[blk][:, sg * 512:(sg + 1) * 512])
                        nc.vector.bn_stats(out=st[:, 4 + sg, :],
                                           in_=xk[blk][:, sg * 512:(sg + 1) * 512])
                    mv = work2.tile([128, 2], F32, tag="bnmv", name=f"bnmv{blk}")
                    nc.vector.bn_aggr(out=mv[:], in_=st[:])
                    s2 = per.tile([128, 2], F32, tag=f"s2_{blk}", name=f"s2_{blk}")
                    nc.vector.tensor_copy(out=s2[:, 0:1], in_=mv[:, 0:1])
                    msq = work2.tile([128, 1], F32, tag="msq", name=f"msq{blk}")
                    nc.vector.tensor_mul(out=msq[:], in0=mv[:, 0:1], in1=mv[:, 0:1])
                    nc.vector.tensor_add(out=s2[:, 1:2], in0=mv[:, 1:2], in1=msq[:])
                    stats2.append(s2)

                gstat_ps = psA.tile([8, 2, 512], F32, tag="s", name="gstat_ps")
                for blk in range(2):
                    nc.tensor.matmul(gstat_ps[:, 0, 0:2], gsb[blk][:], stats2[blk][:],
                                     start=(blk == 0), stop=(blk == 1))
                gstat = per.tile([8, 2], F32, tag="gstat")
                nc.vector.tensor_scalar_mul(out=gstat[:], in0=gstat_ps[:, 0, 0:2],
                                            scalar1=1.0 / GS)
                gmsq = per.tile([8, 1], F32, tag="gmsq")
                nc.vector.tensor_mul(out=gmsq[:], in0=gstat[:, 0:1], in1=gstat[:, 0:1])
                gvar = per.tile([8, 1], F32, tag="gvar")
                nc.vector.tensor_tensor(out=gvar[:], in0=gstat[:, 1:2], in1=gmsq[:],
                                        op=OP.subtract)
                # rstd = exp(-0.5 * ln(var + eps)); Ln+Exp share one ACT table set
                glog = per.tile([8, 1], F32, tag="glog")
                nc.scalar.activation(out=glog[:], in_=gvar[:], func=AF.Ln,
                                     bias=eps8[:], scale=1.0)
                grstd = per.tile([8, 1], F32, tag="grstd")
                nc.scalar.activation(out=grstd[:], in_=glog[:], func=AF.Exp, scale=-0.5)
                ms8 = per.tile([8, 2], F32, tag="ms8")
                nc.vector.tensor_copy(out=ms8[:, 0:1], in_=gstat[:, 0:1])
                nc.vector.tensor_copy(out=ms8[:, 1:2], in_=grstd[:])

                for blk in range(2):
                    bc_ps = psA.tile([128, 2, 512], F32, tag="s", name=f"gnbc{blk}")
                    nc.tensor.matmul(bc_ps[:, 0, 0:2], g0t[blk][:], ms8[:],
                                     start=True, stop=True)
                    A = work2.tile([128, 1], F32, tag="gnA", name=f"gnA{blk}")
                    nc.vector.tensor_mul(out=A[:], in0=bc_ps[:, 0, 1:2],
                                         in1=normw[:, blk:blk + 1])
                    t0 = work2.tile([128, 1], F32, tag="gnT", name=f"gnT{blk}")
                    nc.vector.tensor_mul(out=t0[:], in0=bc_ps[:, 0, 0:1], in1=A[:])
                    Bc = work2.tile([128, 1], F32, tag="gnB", name=f"gnB{blk}")
                    nc.vector.tensor_tensor(out=Bc[:], in0=normb[:, blk:blk + 1],
                                            in1=t0[:], op=OP.subtract)
                    nc.vector.tensor_scalar(out=h[blk][:, 0:NQ], in0=xq[blk][:],
                                            scalar1=A[:], scalar2=Bc[:],
                                            op0=OP.mult, op1=OP.add)
                    nc.vector.tensor_scalar(out=h[blk][:, NQ:N], in0=xk[blk][:],
                                            scalar1=A[:], scalar2=Bc[:],
                                            op0=OP.mult, op1=OP.add)

            # ---------- weight transposes (no x dependency; PE warms early) ----------
            for cb in range(2):
                for t in range(6):
                    tp = psA.tile([128, 2, 512], F32, tag="s", name=f"wq{cb}_{t}")
                    nc.tensor.transpose(tp[:, 0, 0:128],
                                        wstage[:, t, cb * 128:(cb + 1) * 128], ident[:])
                    nc.vector.tensor_copy(out=wtq[cb][:, t * 128:(t + 1) * 128],
                                          in_=tp[:, 0, 0:128])
                for t in range(2):
                    tp = psA.tile([128, 2, 512], F32, tag="s", name=f"wp{cb}_{t}")
                    nc.tensor.transpose(tp[:, 0, 0:128],
                                        pstage[:, t, cb * 128:(cb + 1) * 128], ident[:])
                    nc.vector.tensor_copy(out=wtp[cb][:, t * 128:(t + 1) * 128],
                                          in_=tp[:, 0, 0:128])

            # ---------- qkv / V^T emission helpers ----------
            def q_chunk(hp, ch):
                qp = psC.tile([128, 512], F32, tag="c", name=f"qp{hp}_{ch}")
                for cb in range(2):
                    nc.tensor.matmul(qp[:], wtq[cb][:, hp * 128:(hp + 1) * 128],
                                     h[cb][:, ch * 512:(ch + 1) * 512],
                                     start=(cb == 0), stop=(cb == 1))
                nc.vector.tensor_scalar(out=Q[hp][:, ch * 512:(ch + 1) * 512],
                                        in0=qp[:], scalar1=qkvb[:, hp:hp + 1],
                                        scalar2=None, op0=OP.add)

            def k_chunk(hp, ch):
                kp = psC.tile([128, 512], F32, tag="c", name=f"kp{hp}_{ch}")
                for cb in range(2):
                    nc.tensor.matmul(kp[:],
                                     wtq[cb][:, 256 + hp * 128:256 + (hp + 1) * 128],
                                     h[cb][:, ch * 512:(ch + 1) * 512],
                                     start=(cb == 0), stop=(cb == 1))
                nc.vector.tensor_scalar(out=K[hp][:, ch * 512:(ch + 1) * 512],
                                        in0=kp[:], scalar1=qkvb[:, 2 + hp:3 + hp],
                                        scalar2=None, op0=OP.add)

            def vta_tile(hp, j):
                vp = psC.tile([128, 512], F32, tag="c", name=f"vp{hp}_{j}")
                nc.tensor.matmul(vp[:, 0:128], ones1b[:],
                                 vbias_b[:, hp * 128:(hp + 1) * 128],
                                 start=True, stop=False)
                for cb in range(2):
                    nc.tensor.matmul(vp[:, 0:128], h[cb][:, j * 128:(j + 1) * 128],
                                     wtq[cb][:, 512 + hp * 128:512 + (hp + 1) * 128],
                                     start=False, stop=(cb == 1))
                nc.vector.tensor_copy(out=vta[hp][:, j, 0:64], in_=vp[:, 0:64])
                nc.vector.tensor_copy(out=vta[hp][:, j, 65:129], in_=vp[:, 64:128])

            # ones columns for the softmax-denominator trick (no data deps)
            for hp in range(2):
                nc.vector.tensor_copy(out=vta[hp][:, :, 64:65],
                                      in_=ones_col[:, 0:1, None].to_broadcast((128, 32, 1)))
                nc.vector.tensor_copy(out=vta[hp][:, :, 129:130],
                                      in_=ones_col[:, 0:1, None].to_broadcast((128, 32, 1)))

            # hp=0 prep upfront; hp=1 prep + vta tiles stream in as
            # background work inside the attention loops
            for ch in range(NQ // 512):
                q_chunk(0, ch)
            for ch in range(N // 512):
                k_chunk(0, ch)

            # ---------- attention ----------
            attn = [per.tile([128, NQ], F32R, tag=f"attn{hp}", name=f"attn{hp}")
                    for hp in range(2)]
            # unnormalized softmax row-sums, per hp: row r = qc*2 + h
            sums8 = [per.tile([8, 512], F32, tag=f"sums8_{hp}", name=f"sums8_{hp}")
                     for hp in range(2)]

            def chunk_drain(hp, qc, outp):
                """Move PV accumulators out of PSUM: sums row + unnormalized
                attention columns. Frees outp quickly (psB is single-buffered)."""
                qsl = slice(qc * 512, (qc + 1) * 512)
                for h_ in range(2):
                    r = qc * 2 + h_
                    sm = work2.tile([1, 512], F32, tag="sm", name=f"sm{hp}{qc}{h_}")
                    nc.vector.tensor_copy(out=sm[:], in_=outp[h_][64:65, :])
                    nc.sync.dma_start(out=sums8[hp][r:r + 1, :], in_=sm[:])
                    nc.vector.tensor_copy(out=attn[hp][h_ * 64:(h_ + 1) * 64, qsl],
                                          in_=outp[h_][0:64, :])

            recips = [None, None]

            def batch_recip(hp):
                rc = per.tile([8, 512], F32R, tag=f"recip{hp}", name=f"recip{hp}")
                nc.vector.reciprocal(out=rc[:], in_=sums8[hp][:])
                recips[hp] = rc

            def norm_chunk(hp, qc):
                """attn[hp][:, qc] *= broadcast(recips[hp] rows 2qc, 2qc+1)."""
                qsl = slice(qc * 512, (qc + 1) * 512)
                bcp = psC.tile([128, 512], F32, tag="c", name=f"bcp{hp}{qc}")
                nc.tensor.matmul(bcp[:], eind_r[:, qc * 128:(qc + 1) * 128],
                                 recips[hp][:], start=True, stop=True)
                nc.vector.tensor_tensor(out=attn[hp][:, qsl],
                                        in0=bcp[:], in1=attn[hp][:, qsl],
                                        op=OP.mult)

            def proj_chunk(ch):
                for ob in range(2):
                    pj = psB.tile([128, 512], F32, tag=f"out{ob}", name=f"pj{ob}_{ch}")
                    for cb in range(2):
                        nc.tensor.matmul(pj[:], wtp[cb][:, ob * 128:(ob + 1) * 128],
                                         attn[cb][:, ch * 512:(ch + 1) * 512],
                                         start=(cb == 0), stop=(cb == 1))
                    osb = work2.tile([128, 512], F32, tag="osb", name=f"osb{ob}_{ch}")
                    nc.vector.tensor_scalar(out=osb[:], in0=pj[:],
                                            scalar1=projb[:, ob:ob + 1],
                                            scalar2=None, op0=OP.add)
                    nc.vector.tensor_add(out=osb[:], in0=osb[:],
                                         in1=xq[ob][:, ch * 512:(ch + 1) * 512])
                    nc.sync.dma_start(out=out_d[ob * 128:(ob + 1) * 128,
                                                ch * 512:(ch + 1) * 512],
                                      in_=osb[:])

            bg = [lambda j=j: vta_tile(0, j) for j in range(32)]
            bg2 = ([lambda c=c: q_chunk(1, c) for c in range(NQ // 512)]
                   + [lambda c=c: k_chunk(1, c) for c in range(N // 512)]
                   + [lambda j=j: vta_tile(1, j) for j in range(32)])
            pending = None
            for hp in range(2):
                for qc in range(NQ // 512):
                    qsl = slice(qc * 512, (qc + 1) * 512)
                    outp = [psB.tile([128, 512], F32, tag=f"out{h_}",
                                     name=f"o{hp}{qc}{h_}")
                            for h_ in range(2)]
                    pbs = [None] * 32
                    for j in range(33):
                        if bg and (hp == 0 and qc == 0 or j % 2 == 0):
                            bg.pop(0)()
                        if j == 8 and pending is not None:
                            chunk_drain(*pending)
                            pending = None
                        if hp == 1 and j == 12:
                            if qc == 0:
                                batch_recip(0)
                            else:
                                norm_chunk(0, qc - 1)
                        if hp == 1 and j == 20 and qc == 3:
                            norm_chunk(0, 3)
                        if j < 32:
                            sp = psA.tile([128, 2, 512], F32, tag="s",
                                          name=f"sp{hp}_{qc}_{j}")
                            nc.tensor.matmul(sp[:, 0], K[hp][0:64, j * 128:(j + 1) * 128],
                                             Q[hp][0:64, qsl], start=True, stop=True,
                                             tile_position=(0, 0))
                            nc.tensor.matmul(sp[:, 1], K[hp][64:128, j * 128:(j + 1) * 128],
                                             Q[hp][64:128, qsl], start=True, stop=True,
                                             tile_position=(64, 0))
                            pb = work3.tile([128, 2, 512], BF16, tag="p",
                                            name=f"pb{hp}_{qc}_{j}")
                            nc.scalar.activation(out=pb[:], in_=sp[:], func=AF.Exp,
                                                 scale=SCALE)
                            pbs[j] = pb
                        if j > 0:
                            nc.tensor.matmul(outp[0][0:65], vta[hp][:, j - 1, 0:65],
                                             pbs[j - 1][:, 0],
                                             start=(j == 1), stop=(j == 32))
                            nc.tensor.matmul(outp[1][0:65], vta[hp][:, j - 1, 65:130],
                                             pbs[j - 1][:, 1],
                                             start=(j == 1), stop=(j == 32))
                    pending = (hp, qc, outp)
                    if hp == 0 and qc == 0:
                        bg = bg2
            chunk_drain(*pending)
            batch_recip(1)
            for qc in range(NQ // 512):
                norm_chunk(1, qc)
            for ch in range(NQ // 512):
                proj_chunk(ch)

            if dbg:
                nc.sync.dma_start(out=dbg_d["d_h0"], in_=h[0][:].bitcast(mybir.dt.uint16))
                nc.sync.dma_start(out=dbg_d["d_Q0"], in_=Q[0][:].bitcast(mybir.dt.uint16))
                nc.sync.dma_start(out=dbg_d["d_K0"], in_=K[0][:].bitcast(mybir.dt.uint16))
                nc.sync.dma_start(out=dbg_d["d_attn0"], in_=attn[0][:].bitcast(F32))

    _split_waits(nc)
    return nc


def _split_waits(nc):
    """walrus in this container caps sync waits at 1/instruction; hoist
    extras onto preceding single-wait NOPs on the same engine."""
    n_new = 0
    for f in nc.m.functions:
        for bb in f.blocks:
            out = []
            changed = False
            for inst in bb.instructions:
                si = inst.sync_info
                if si is not None and si.on_wait and len(si.on_wait) > 1:
                    waits = list(si.on_wait)
                    for w in waits[:-1]:
                        nop = mybir.InstNoOp(
                            name=f"{inst.name}-ws{n_new}", engine=inst.engine,
                            ins=[], outs=[],
                            sync_info=mybir.SyncInfo(on_wait=[w], on_update=[]),
                        )
                        n_new += 1
                        out.append(nop)
                    si.on_wait = [waits[-1]]
                    changed = True
                out.append(inst)
            if changed:
                bb.instructions = out
    return n_new


def _get_nc(dbg=False):
    key = ("nc", dbg)
    if key not in _CACHE:
        _CACHE[key] = _build(dbg)
    return _CACHE[key]


def _gind():
    g = np.zeros((2, 8, 128), dtype=np.float32)
    for blk in range(2):
        for gr in range(4):
            g[blk, blk * 4 + gr, gr * GS:(gr + 1) * GS] = 1.0
    return g


def _eind():
    e = np.zeros((8, 8, 128), dtype=np.float32)
    for k in range(4):
        e[k, 2 * k, 0:64] = 1.0
        e[k, 2 * k + 1, 64:128] = 1.0
    return np.ascontiguousarray(e.transpose(1, 0, 2).reshape(8, 8 * 128))


def _core_maps(x, norm_w, norm_b, qkv_w, qkv_b, proj_w, proj_b):
    x = np.ascontiguousarray(np.asarray(x, dtype=np.float32)).reshape(B, C, N)
    maps = []
    shared = {
        "qkv_w": np.ascontiguousarray(np.asarray(qkv_w, dtype=np.float32)),
        "qkv_b": np.asarray(qkv_b, dtype=np.float32).reshape(3 * C, 1),
        "proj_w": np.ascontiguousarray(np.asarray(proj_w, dtype=np.float32)),
        "proj_b": np.asarray(proj_b, dtype=np.float32).reshape(C, 1),
        "norm_w": np.asarray(norm_w, dtype=np.float32).reshape(C, 1),
        "norm_b": np.asarray(norm_b, dtype=np.float32).reshape(C, 1),
        "gind": _gind(),
        "gind_t": np.ascontiguousarray(_gind().transpose(0, 2, 1)),
        "eind": _eind(),
    }
    for core in range(NCORES):
        b, half = core // 2, core % 2
        xb = x[b]
        if half == 0:
            xc = xb
        else:
            xc = np.concatenate([xb[:, NQ:], xb[:, :NQ]], axis=1)
        m = dict(shared)
        m["x"] = np.ascontiguousarray(xc)
        maps.append(m)
    return maps


def _run(inputs, trace=False, dbg=False):
    nc = _get_nc(dbg)
    maps = _core_maps(**inputs)
    res = run_bass_kernel_spmd(nc, maps, list(range(NCORES)), trace=trace)
    full = np.empty((B, C, N), dtype=np.float32)
    for core in range(NCORES):
        b, half = core // 2, core % 2
        full[b][:, half * NQ:(half + 1) * NQ] = res.results[core]["out"]
    return full.reshape(B, C, 64, 64), res


def kernel(**inputs):
    out, _ = _run(inputs, trace=False)
    return out
